# revision 53
# baseline (speedup 1.0000x reference)
"""Multi-head attention (B=4, S=2048, E=1024, H=16, D=64) on 8 trn2 cores.

Sharding: core c -> (batch b = c//2, head-group hg = c%2 of 8 heads).
Each core computes its 8 heads' attention for its batch plus the partial
output projection (its 512 rows of w_proj); the host sums the two partials
per batch and adds the folded bias (b_proj + b_v @ w_proj).

Measured-on-HW design notes (For_i-delta microbenchmarks):
  - a lone N=512 matmul group costs ~420 ns; long psum accumulation chains
    ~210-240 ns/matmul; a row-split (rows 0:64 / 64:128) pair of 64-contract
    matmuls runs CONCURRENTLY in the PE array (~270 ns/pair). exp on ACT is
    ~1.2 us per [128,1024]. Cross-engine semaphore latency is ~0.4-0.5 us,
    so every producer->consumer edge needs >= 2 pipeline steps of slack.
  - all matmul operands bf16 (enables fast weight load, halves DMA).
  - all host arrays pre-packed to contiguous per-partition layout; strided
    gather DMAs are descriptor-dominated (8x slower). X rides the SP HWDGE
    ring, weights the ACT ring, SBUF-SBUF moves the SWDGE ring.
  - head-pair loop OUTER, q-tile inner; QKV of pairs 1-3 is woven into the
    pair-0 attention stream as deadline-scheduled 8-matmul groups.
  - P@V trails scores/exp by TWO k-chunk steps (sem latency), epilogue +
    output projection are deferred work units spread one-per-step into the
    following pair's loop, so the in-order PE queue never blocks on DVE.
  - scores/QKV-groups/broadcast/projection share one 3-slot [128,1024] psum
    pool (6 banks); PV accumulators use the other 2 banks.

Layout:
  - x pre-transposed on host to xT [E, S] (e-major).
  - Q^T, K^T d-major [128(d of pair), 4(pair), S]; scores^T chunks
    [k=128, q=512] come straight out of matmul; the two heads of a pair run
    on distinct PE row groups concurrently.
  - V s-major [128(s), kc, head, 65] with col 64 = 1.0 so P@V accumulates
    softmax denominators in psum row 64 for free.
  - exp() uses no max-subtraction: scores here are O(1), far from overflow.
  - K bias is dropped entirely: q.bk is constant along k -> cancels in
    softmax. V bias is folded into b_proj on the host.
  - attention output lands transposed [d, q] = the lhsT layout the output
    projection needs.
"""

import ml_dtypes
import numpy as np

S = 2048
E = 1024
NCORES = 8

_PROGRAM = None
TRACE = False
LAST_RESULT = None
# timing-ablation switch used by hw_time.py only (never set in grading)
ABLATE = None


def _build_body(tc, t, o, s_len):
    import concourse.bass as bass  # noqa: F401
    from concourse import mybir

    nc = tc.nc
    f32 = mybir.dt.float32
    f32r = mybir.dt.float32r
    bf16 = mybir.dt.bfloat16
    AF = mybir.ActivationFunctionType
    ALU = mybir.AluOpType

    ST = s_len // 512   # number of 512-wide s/q tiles
    KC = s_len // 128   # number of 128-row k chunks

    def r(ap):
        return ap.bitcast(f32r)

    with tc.tile_pool(name="const", bufs=1) as constp, \
         tc.tile_pool(name="big", bufs=1) as bigp, \
         tc.tile_pool(name="at", bufs=6) as atp, \
         tc.tile_pool(name="iv", bufs=2) as ivp, \
         tc.tile_pool(name="ob", bufs=2) as obp, \
         tc.tile_pool(name="ps", bufs=3, space="PSUM") as psp, \
         tc.tile_pool(name="ot", bufs=2, space="PSUM") as otp:

        # X lives as 4 s-stripe tiles so stripe 0 (1 MB, ~4us) unblocks the
        # first QKV groups long before the full 4 MB lands
        XS = [bigp.tile([128, 8, 512], bf16, name=f"XS{st}")
              for st in range(ST)]
        QT = bigp.tile([128, 4, s_len], bf16, name="QT")
        KT = bigp.tile([128, 4, s_len], bf16, name="KT")
        V = bigp.tile([128, KC, 8, 65], bf16, name="V")
        WQ = bigp.tile([128, 8, 512], bf16, name="WQ")
        WK = bigp.tile([128, 8, 512], bf16, name="WK")
        WV = bigp.tile([128, 8, 512], bf16, name="WV")
        WP = bigp.tile([128, 4, 1024], bf16, name="WP")
        HT = bigp.tile([128, 4, s_len], bf16, name="HT")
        CONSTS = constp.tile([128, 8], f32, name="CONSTS")
        BQ = CONSTS[:, 0:4]
        ONES = constp.tile([128, 64], f32, name="ONES")
        ONESB = constp.tile([128, 128], bf16, name="ONESB")

        # input DMAs: X on the SP HWDGE ring, weights/consts on the ACT ring
        for st in range(ST):
            nc.sync.dma_start(XS[st], t["xr"][:, st, :, :])
        nc.scalar.dma_start(WQ, t["wq"])
        nc.scalar.dma_start(WK, t["wk"])
        nc.scalar.dma_start(CONSTS, t["consts"])
        nc.scalar.dma_start(r(ONES), r(t["ones"]))
        nc.scalar.dma_start(WV, t["wv"])
        nc.scalar.dma_start(ONESB, t["onesb"])
        # V's 65th column of ones via DVE (a scattered DMA of 16K 1-byte
        # elements is descriptor hell); col 65 is never-read padding
        nc.vector.tensor_copy(
            V[:, :, :, 64],
            ONESB[:, 0:KC * 8].rearrange("p (c h) -> p c h", h=8),
        )
        nc.scalar.dma_start(WP, t["wp"])

        if ABLATE == "dmaonly":
            dummy = obp.tile([128, 4, 1024], f32, name="ob")
            nc.vector.tensor_copy(dummy[:, 0, 0:512], XS[0][:, 0, :])
            nc.vector.tensor_copy(dummy[:, 1, 0:512], WQ[:, 0, :])
            nc.vector.tensor_copy(dummy[:, 2, 0:512], WK[:, 0, :])
            nc.vector.tensor_copy(dummy[:, 3, 0:512], WV[:, 0, :])
            nc.vector.tensor_copy(dummy[:, 0, 512:1024], WP[:, 0, 0:512])
            nc.vector.tensor_copy(dummy[:, 1, 512:520], V[:, 0, :, 64])
            nc.vector.tensor_copy(dummy[:, 2, 512:520], CONSTS)
            nc.vector.tensor_copy(dummy[:, 3, 512:576], ONES)
            nc.sync.dma_start(
                o[0:512, :].rearrange("(q p) e -> p q e", p=128), dummy)
            return

        # ---------- phase-1 groups (8-matmul psum accumulations; a group
        # holds its 2-bank psum slot ~1.9us, leaving the scores rotation its
        # two slots) ----------
        def q_group(j, st):
            ss = slice(st * 512, (st + 1) * 512)
            qp = psp.tile([128, 1024], f32, name="ps")
            for c in range(8):
                nc.tensor.matmul(
                    qp[:, 0:512], WQ[:, c, j * 128:(j + 1) * 128], XS[st][:, c, :],
                    start=(c == 0), stop=(c == 7),
                )
            # QT = 0.125 * (x@wq) + 0.125*bq   (bq pre-scaled on host)
            nc.vector.tensor_scalar(
                QT[:, j, ss], qp[:, 0:512], 0.125, BQ[:, j:j + 1],
                ALU.mult, ALU.add,
            )

        def k_group(j, st):
            ss = slice(st * 512, (st + 1) * 512)
            kp = psp.tile([128, 1024], f32, name="ps")
            for c in range(8):
                nc.tensor.matmul(
                    kp[:, 0:512], WK[:, c, j * 128:(j + 1) * 128], XS[st][:, c, :],
                    start=(c == 0), stop=(c == 7),
                )
            # no K bias: q.bk is constant along k, cancels in softmax
            nc.vector.tensor_copy(KT[:, j, ss], kp[:, 0:512])

        def v_group(kc):
            vp = psp.tile([128, 1024], f32, name="ps")
            st, so = divmod(kc * 128, 512)
            for c in range(8):
                nc.tensor.matmul(
                    vp[:, 0:512], XS[st][:, c, so:so + 128], WV[:, c, :],
                    start=(c == 0), stop=(c == 7),
                )
            nc.vector.tensor_copy(
                V[:, kc, :, 0:64],
                vp[:, 0:512].rearrange("p (h d) -> p h d", d=64),
            )

        # deadline-ordered queue: attention step (j*4+qt)*16+tt consumes
        # Q[j][qt] (dl j*64+qt*16), K[j][st] (dl j*64+4*st), V[kc] (dl kc)
        pend = []
        seq = 0
        for kc in range(KC):
            pend.append((kc, seq, lambda kc=kc: v_group(kc)))
            seq += 1
        for j in range(4):
            for st in range(ST):
                pend.append((j * 64 + 4 * st, seq,
                             lambda j=j, st=st: k_group(j, st)))
                seq += 1
                pend.append((j * 64 + st * 16, seq,
                             lambda j=j, st=st: q_group(j, st)))
                seq += 1
        pend.sort(key=lambda x: (x[0], x[1]))
        pidx = [0]

        def ensure(step):
            while pidx[0] < len(pend) and pend[pidx[0]][0] <= step:
                pend[pidx[0]][2]()
                pidx[0] += 1

        def pace():
            if pidx[0] < len(pend):
                pend[pidx[0]][2]()
                pidx[0] += 1

        if ABLATE == "qkv":
            while pidx[0] < len(pend):
                pace()
            dummy = obp.tile([128, 4, 1024], f32, name="ob")
            nc.vector.tensor_copy(dummy[:, 0, 0:512], QT[:, 0, 0:512])
            nc.vector.tensor_copy(dummy[:, 1, 0:512], KT[:, 0, 0:512])
            nc.vector.tensor_copy(
                dummy[:, 2, 0:520], V[:, 0, :, :].rearrange("p h d -> p (h d)"))
            nc.sync.dma_start(
                o[0:512, :].rearrange("(q p) e -> p q e", p=128), dummy)
            return

        # ---------- attention + output projection ----------
        # deferred work units: emitted one per k-chunk step starting at step
        # 4 of the FOLLOWING (j, qt) loop, so the PE-queue instructions they
        # contain never head-of-line block on fresh DVE results
        deferred = []

        bslot = [0]

        def bcast_dram_unit(st8):
            # stage the 1/denominator row to a DRAM scratch slot (SWDGE ring)
            j, qs, qt, oA, oB, ivAB, bcs = st8
            slot = bslot[0] % 2
            nc.gpsimd.dma_start(t["bsc"][slot:slot + 1, :], ivAB[64:65, :])

        def bcast_unit(st8):
            # replicate it to partitions 0:64 with a stride-0-partition DMA
            # from DRAM (zero PE/DVE cost; the broadcast matmul + psum copy
            # it replaces cost ~1.5us of PE<->DVE ping-pong per pair). Both
            # transfers ride the same SWDGE ring, whose FIFO orders them.
            j, qs, qt, oA, oB, ivAB, bcs = st8
            slot = bslot[0] % 2
            bslot[0] += 1
            src = t["bsc"][slot:slot + 1, :]
            nc.gpsimd.dma_start(
                bcs,
                bass.AP(tensor=src.tensor, offset=src.offset,
                        ap=[[0, 64]] + list(src.ap[1:])),
            )

        def mul_unit(st8):
            j, qs, qt, oA, oB, ivAB, bcs = st8
            # head A: all operands at partitions 0:64
            nc.vector.tensor_mul(HT[0:64, j, qs], oA[0:64, :], bcs[:, 0:512])
            # head B: compute at base 0, DMA-move to partitions 64:128 on
            # the SWDGE ring
            stg = ivp.tile([64, 512], bf16, name="stg")
            nc.vector.tensor_mul(stg, oB[0:64, :], bcs[:, 512:1024])
            nc.gpsimd.dma_start(HT[64:128, j, qs], stg)

        proj_psum = []

        def proj_mm_unit(qt, q4):
            rs = slice(qt * 512 + q4 * 128, qt * 512 + (q4 + 1) * 128)
            pj = psp.tile([128, 1024], f32, name="ps")
            for half in range(2):
                for c in range(4):
                    nc.tensor.matmul(
                        pj[:, half * 512:(half + 1) * 512], HT[:, c, rs],
                        WP[:, c, half * 512:(half + 1) * 512],
                        start=(c == 0), stop=(c == 3),
                    )
            proj_psum.append(pj)

        def proj_cp_unit(q4, ob):
            nc.vector.tensor_copy(ob[:, q4, :], proj_psum.pop(0))

        def out_unit(qt, ob):
            nc.sync.dma_start(
                o[qt * 512:(qt + 1) * 512, :].rearrange(
                    "(q p) e -> p q e", p=128),
                ob,
            )

        def queue_pair_epilogue(st8):
            j, qs, qt, oA, oB, ivAB, bcs = st8
            if j != 3:
                deferred.append(lambda: bcast_dram_unit(st8))
                deferred.append(lambda: bcast_unit(st8))
                deferred.append(lambda: mul_unit(st8))
                return
            # j3 windows also carry the output projection for this q tile.
            # pj matmuls (PE-only) and psum->ob copies (DVE-only) are spaced
            # >= 2 slots apart so neither engine queue ever waits on a fresh
            # cross-engine result; at most one pj psum tile is live, leaving
            # the scores rotation its two slots.
            ob = obp.tile([128, 4, 1024], f32, name="ob")
            deferred.append(lambda: bcast_dram_unit(st8))
            deferred.append(lambda: bcast_unit(st8))
            deferred.append(lambda: mul_unit(st8))
            deferred.append(lambda: proj_mm_unit(qt, 0))
            deferred.append(lambda: None)
            deferred.append(lambda: proj_cp_unit(0, ob))
            deferred.append(lambda: proj_mm_unit(qt, 1))
            deferred.append(lambda: None)
            deferred.append(lambda: proj_cp_unit(1, ob))
            deferred.append(lambda: proj_mm_unit(qt, 2))
            deferred.append(lambda: None)
            deferred.append(lambda: proj_cp_unit(2, ob))
            deferred.append(lambda: proj_mm_unit(qt, 3))
            deferred.append(lambda: None)
            deferred.append(lambda: proj_cp_unit(3, ob))
            deferred.append(lambda: out_unit(qt, ob))

        step = 0
        for j in range(4):  # head pairs; A = head 2j, B = head 2j+1
            for qt in range(ST):
                qs = slice(qt * 512, (qt + 1) * 512)
                outA = otp.tile([128, 512], f32, name="ot")
                outB = otp.tile([128, 512], f32, name="ot")

                def emit_pv(at_t, ttp):
                    nc.tensor.matmul(
                        outA[0:65, :], V[:, ttp, 2 * j, :], at_t[:, 0:512],
                        start=(ttp == 0), stop=(ttp == KC - 1),
                    )
                    nc.tensor.matmul(
                        outB[0:65, :], V[:, ttp, 2 * j + 1, :], at_t[:, 512:1024],
                        start=(ttp == 0), stop=(ttp == KC - 1),
                    )

                # software pipeline: PV trails scores/exp by TWO steps so the
                # ~0.4us cross-engine sem latency never lands on the critical
                # scores->exp edge
                pipe = []
                for tt in range(KC):
                    ensure(step)
                    if step % 3 == 0:
                        pace()
                    if tt >= 4 and deferred:
                        deferred.pop(0)()
                    ks = slice(tt * 128, (tt + 1) * 128)
                    sc = psp.tile([128, 1024], f32, name="ps")
                    nc.tensor.matmul(
                        sc[:, 0:512], KT[0:64, j, ks], QT[0:64, j, qs],
                        start=True, stop=True,
                    )
                    nc.tensor.matmul(
                        sc[:, 512:1024], KT[64:128, j, ks], QT[64:128, j, qs],
                        start=True, stop=True,
                    )
                    at = atp.tile([128, 1024], bf16, name="at")
                    if ABLATE == "smallexp":
                        nc.scalar.activation(at[:, 0:128], sc[:, 0:128], AF.Exp)
                        nc.scalar.activation(at[:, 512:640], sc[:, 512:640], AF.Exp)
                    else:
                        nc.scalar.activation(at, sc, AF.Exp)
                    pipe.append(at)
                    if ABLATE != "noattout" and len(pipe) > 2:
                        emit_pv(pipe.pop(0), tt - 2)
                    step += 1
                if ABLATE == "noattout":
                    dm = ivp.tile([65, 512], f32, name="oA")
                    nc.vector.tensor_copy(dm[:, :], pipe[-1][0:65, 0:512])
                    continue
                emit_pv(pipe.pop(0), KC - 2)
                emit_pv(pipe.pop(0), KC - 1)
                # copy accumulators out of psum immediately so the next
                # pair's PV can claim the banks, and take the reciprocals
                # right away (pure DVE, nothing waits on them yet)
                oA = ivp.tile([65, 512], f32, name="oA")
                oB = ivp.tile([65, 512], f32, name="oB")
                nc.vector.tensor_copy(oA, outA[0:65, :])
                nc.vector.tensor_copy(oB, outB[0:65, :])
                if ABLATE == "noepi":
                    continue
                ivAB = ivp.tile([65, 1024], f32, name="ivAB")
                with nc.allow_low_precision(reason="softmax denom in f32r"):
                    nc.vector.reciprocal(r(ivAB[64:65, 0:512]), oA[64:65, :])
                    nc.vector.reciprocal(r(ivAB[64:65, 512:1024]), oB[64:65, :])
                bcs = ivp.tile([64, 1024], f32, name="bcs")
                queue_pair_epilogue((j, qs, qt, oA, oB, ivAB, bcs))
        while deferred:
            deferred.pop(0)()
        if ABLATE in ("noattout", "noepi"):
            dummy = obp.tile([128, 4, 1024], f32, name="ob")
            nc.vector.tensor_copy(dummy[:, 0, 0:512], QT[:, 0, 0:512])
            nc.sync.dma_start(
                o[0:512, :].rearrange("(q p) e -> p q e", p=128), dummy)


def _build_program(s_len=S, repeat=1):
    import concourse.bacc as bacc
    import concourse.tile as tile
    from concourse import mybir

    f32 = mybir.dt.float32
    bf16 = mybir.dt.bfloat16
    nc = bacc.Bacc(
        "TRN2", target_bir_lowering=False, debug=False, num_devices=NCORES
    )
    t = {
        "xr": nc.dram_tensor("xr", [128, 4, 8, 512], bf16, kind="ExternalInput").ap(),
        "wq": nc.dram_tensor("wq", [128, 8, 512], bf16, kind="ExternalInput").ap(),
        "wk": nc.dram_tensor("wk", [128, 8, 512], bf16, kind="ExternalInput").ap(),
        "wv": nc.dram_tensor("wv", [128, 8, 512], bf16, kind="ExternalInput").ap(),
        "wp": nc.dram_tensor("wp", [128, 4, E], bf16, kind="ExternalInput").ap(),
        "consts": nc.dram_tensor(
            "consts", [128, 8], f32, kind="ExternalInput"
        ).ap(),
        "ones": nc.dram_tensor("ones", [128, 64], f32, kind="ExternalInput").ap(),
        "onesb": nc.dram_tensor(
            "onesb", [128, 128], bf16, kind="ExternalInput"
        ).ap(),
    }
    o = nc.dram_tensor("o", [s_len, E], f32, kind="ExternalOutput").ap()
    t["bsc"] = nc.dram_tensor("bsc", [2, 1024], f32, kind="Internal").ap()
    with tile.TileContext(nc) as tc:
        if repeat > 1:
            # timing harness: run the whole body in a hardware loop so device
            # time dominates wall-clock (amortizes transfer/dispatch)
            with tc.For_i(0, repeat, 1):
                _build_body(tc, t, o, s_len)
        else:
            _build_body(tc, t, o, s_len)
    nc.compile()
    return nc


def _get_program():
    global _PROGRAM
    if _PROGRAM is None:
        _PROGRAM = _build_program()
    return _PROGRAM


def _shard_inputs(x, w_qkv, b_qkv, w_proj):
    bf16 = ml_dtypes.bfloat16

    def pack(a, nchunk):
        # [nchunk*128, F] -> [128, nchunk, F]  (e = c*128 + p)
        return np.ascontiguousarray(
            a.reshape(nchunk, 128, a.shape[1]).transpose(1, 0, 2)
        ).astype(bf16)

    wq_f, wk_f, wv_f = w_qkv[:, :E], w_qkv[:, E:2 * E], w_qkv[:, 2 * E:]
    bq_f = b_qkv[:E]
    in_maps = []
    for c in range(NCORES):
        b, hg = divmod(c, 2)
        sl = slice(hg * 512, (hg + 1) * 512)
        consts = np.zeros((128, 8), np.float32)
        consts[:, 0:4] = (bq_f[sl] * 0.125).reshape(4, 128).T
        in_maps.append({
            "xr": np.ascontiguousarray(
                x[b].T.reshape(8, 128, 4, 512).transpose(1, 2, 0, 3)
            ).astype(bf16),
            "wq": pack(wq_f[:, sl], 8),
            "wk": pack(wk_f[:, sl], 8),
            "wv": pack(wv_f[:, sl], 8),
            "wp": pack(w_proj[sl, :], 4),
            "consts": consts,
            "ones": np.ones((128, 64), np.float32),
            "onesb": np.ones((128, 128), bf16),
        })
    return in_maps


def kernel(x, w_qkv, b_qkv, w_proj, b_proj):
    global LAST_RESULT
    from concourse.bass_utils import run_bass_kernel_spmd

    x = np.asarray(x, dtype=np.float32)
    w_qkv = np.asarray(w_qkv, dtype=np.float32)
    b_qkv = np.asarray(b_qkv, dtype=np.float32)
    w_proj = np.asarray(w_proj, dtype=np.float32)
    b_proj = np.asarray(b_proj, dtype=np.float32)

    nc = _get_program()
    in_maps = _shard_inputs(x, w_qkv, b_qkv, w_proj)
    res = run_bass_kernel_spmd(nc, in_maps, list(range(NCORES)), trace=TRACE)
    LAST_RESULT = res

    bv_f = b_qkv[2 * E:]
    b_eff = (b_proj + bv_f @ w_proj).astype(np.float32)
    out = np.empty((4, S, E), dtype=np.float32)
    for b in range(4):
        out[b] = res.results[2 * b]["o"] + res.results[2 * b + 1]["o"] + b_eff
    return out
